# revision 1
# baseline (speedup 1.0000x reference)
"""Causal multi-head attention with RoPE on 8 Trainium2 NeuronCores.

Sharding: core = (batch b, head-group hg): b = core//4, hg = core%4.
Each core computes 4 heads of one batch element end-to-end (QKV projection,
RoPE, causal softmax attention, output-projection partial) and the host sums
the 4 per-head-group partials per batch (the "all-reduce" of the O-proj).

Device-side layout choices (per core):
  xT   [1024, 2048]  x[b] transposed (d on partitions, 8 chunks of 128)
  q^T,k^T  [256, 2048] head-transposed, computed with W^T stationary
  v    [l, e] layout (bf16) with a ones column appended per head
  scores computed transposed: S^T[k, q] = k^T.T @ q^T in f32r (TF32), exp
  with no max subtraction (scores bounded ~17), causal mask applied
  multiplicatively post-exp in bf16, AV matmul in bf16 gives O^T[e+1, q]
  whose last row is the softmax denominator. Reciprocals batched per
  q-chunk on DVE; broadcast across partitions on the idle GpSimd engine.
Score-path matmuls run in float32r (TF32, full PE rate for moving dims
>= 256); the P*V path runs in bf16 (P in [0,1]-scale, errors cancel
between numerator and denominator).
"""

import numpy as np

_B, _L, _D, _H, _HD = 2, 2048, 1024, 16, 64
_HPG = 4              # heads per group (per core)
_EG = _HPG * _HD      # 256
_NCORES = 8
_THETA = 10000.0
_QC = 512             # q-chunk width
_NQC = _L // _QC      # 4
_GK = 2               # k-tiles (128) per exp group
_NKC = _D // 128      # 8 contraction chunks for projections
_LC = 512             # l-chunk for phase 1
_NLC = _L // _LC

_CACHE = {}


def _tf32(a):
    """Round float32 array to TF32 (fp32r): RNE to 10-bit mantissa."""
    b = np.ascontiguousarray(a, dtype=np.float32).view(np.uint32)
    b = (b + np.uint32(0xFFF) + ((b >> np.uint32(13)) & np.uint32(1))) \
        & np.uint32(0xFFFFE000)
    return b.view(np.float32)


def _build_nc(debug_taps=False):
    from contextlib import ExitStack

    import concourse.mybir as mybir
    import concourse.tile as tile
    from concourse import bacc

    f32 = mybir.dt.float32
    f32r = mybir.dt.float32r
    bf16 = mybir.dt.bfloat16
    EXP = mybir.ActivationFunctionType.Exp

    nc = bacc.Bacc("TRN2", target_bir_lowering=False, debug=False,
                   enable_asserts=False)
    xT = nc.dram_tensor("xT", [_D, _L], f32r, kind="ExternalInput")
    wq = nc.dram_tensor("wq", [_D, _EG], f32r, kind="ExternalInput")
    wk = nc.dram_tensor("wk", [_D, _EG], f32r, kind="ExternalInput")
    wv = nc.dram_tensor("wv", [_D, _EG], f32r, kind="ExternalInput")
    wo = nc.dram_tensor("wo", [_EG, _D], f32r, kind="ExternalInput")
    cs = nc.dram_tensor("cs", [128, _L], f32, kind="ExternalInput")
    sn = nc.dram_tensor("sn", [128, _L], f32, kind="ExternalInput")
    msk = nc.dram_tensor("msk", [128, _QC // 128, _QC], bf16,
                         kind="ExternalInput")
    perm = nc.dram_tensor("perm", [128, 128], f32r, kind="ExternalInput")
    vones = nc.dram_tensor("vones", [128, _HD], bf16, kind="ExternalInput")
    onesr = nc.dram_tensor("onesr", [128, _HD], f32r, kind="ExternalInput")
    y = nc.dram_tensor("y", [_L, _D], f32, kind="ExternalOutput")
    taps = {}
    if debug_taps:
        taps["dq"] = nc.dram_tensor("dq", [128, 2, _L], f32r,
                                    kind="ExternalOutput")
        taps["dk"] = nc.dram_tensor("dk", [128, 2, _L], f32r,
                                    kind="ExternalOutput")
        taps["dv"] = nc.dram_tensor("dv", [128, _L // 128, _HPG, _HD + 1],
                                    bf16, kind="ExternalOutput")
        taps["dden"] = nc.dram_tensor("dden", [_NQC, _HPG * _QC], f32,
                                      kind="ExternalOutput")
        taps["drc"] = nc.dram_tensor("drc", [_NQC, _HPG * _QC], f32,
                                     kind="ExternalOutput")
        taps["doT"] = nc.dram_tensor("doT", [128, 2, _L], f32r,
                                     kind="ExternalOutput")
        taps["dpt"] = nc.dram_tensor("dpt", [128, _GK * _QC], bf16,
                                     kind="ExternalOutput")

    with tile.TileContext(nc) as tc, ExitStack() as ctx:
        persist = ctx.enter_context(tc.tile_pool(name="persist", bufs=1))
        qT_sb = persist.tile([128, 2, _L], f32r)
        kT_sb = persist.tile([128, 2, _L], f32r)
        v_sb = persist.tile([128, _L // 128, _HPG, _HD + 4], bf16)
        oT_sb = persist.tile([128, 2, _L], f32r)
        wo_sb = persist.tile([128, 2, _D], f32r)
        msk_sb = persist.tile([128, _QC // 128, _QC], bf16)
        ones_sb = persist.tile([128, _HD], f32r)
        wq_sb = persist.tile([128, _NKC, _EG], f32r)
        wk_sb = persist.tile([128, _NKC, _EG], f32r)
        wv_sb = persist.tile([128, _NKC, _EG], f32r)
        cs_sb = persist.tile([128, _L], f32)
        sn_sb = persist.tile([128, _L], f32)
        perm_sb = persist.tile([128, 128], f32r)

        xtp = ctx.enter_context(tc.tile_pool(name="xtp", bufs=2))
        rtmp = ctx.enter_context(tc.tile_pool(name="rtmp", bufs=2))
        ptp = ctx.enter_context(tc.tile_pool(name="ptp", bufs=4))
        nrm = ctx.enter_context(tc.tile_pool(name="nrm", bufs=2))
        otcp = ctx.enter_context(tc.tile_pool(name="otc", bufs=6))
        # PSUM budget (8 banks): sps 2x2 + ops 2x1 + scr 2x1
        sps = ctx.enter_context(tc.tile_pool(name="sps", bufs=2, space="PSUM"))
        ops = ctx.enter_context(tc.tile_pool(name="ops", bufs=2, space="PSUM"))
        scr = ctx.enter_context(tc.tile_pool(name="scr", bufs=2, space="PSUM"))

        # --- input loads: weights split per contraction chunk so compute
        # starts as soon as the first slices land ---
        wq_r = wq.rearrange("(c p) e -> p c e", p=128)
        wk_r = wk.rearrange("(c p) e -> p c e", p=128)
        wv_r = wv.rearrange("(c p) e -> p c e", p=128)
        xT_r = xT.rearrange("(c p) l -> p c l", p=128)
        for kc in range(_NKC):
            nc.sync.dma_start(out=wq_sb[:, kc, :], in_=wq_r[:, kc, :])
            nc.sync.dma_start(out=wk_sb[:, kc, :], in_=wk_r[:, kc, :])
        xts = {}
        for lc in range(2):
            xt = xts[lc] = xtp.tile([128, _NKC, _LC], f32r, tag="xt",
                                    name=f"xt{lc}")
            for kc in range(_NKC):
                nc.sync.dma_start(
                    out=xt[:, kc, :],
                    in_=xT_r[:, kc, lc * _LC:(lc + 1) * _LC])
        nc.gpsimd.dma_start(out=perm_sb, in_=perm[:, :])
        nc.gpsimd.dma_start(out=cs_sb, in_=cs[:, :])
        nc.gpsimd.dma_start(out=sn_sb, in_=sn[:, :])
        for kc in range(_NKC):
            nc.sync.dma_start(out=wv_sb[:, kc, :], in_=wv_r[:, kc, :])
        nc.gpsimd.dma_start(out=ones_sb, in_=onesr[:, :])
        nc.gpsimd.dma_start(out=msk_sb, in_=msk[:, :, :])
        nc.gpsimd.dma_start(out=wo_sb,
                            in_=wo.rearrange("(c p) d -> p c d", p=128))
        nc.gpsimd.dma_start(
            out=v_sb[:, :, :, _HD:_HD + 1],
            in_=vones.rearrange("p (a b) -> p a b",
                                a=_L // 128).unsqueeze(3))

        def norm_head(st, h):
            """broadcast recip + normalize + place head h of chunk st."""
            qc, qs, otcs, drow = st["qc"], st["qs"], st["otcs"], st["drow"]
            c, pb = h // 2, 64 * (h % 2)
            # rank-1 broadcast: ones[1,64].T @ recip_row -> [64, 512]
            bc = scr.tile([128, _QC], f32, tag="scr", name=f"bc{qc}_{h}")
            bc = bc[0:64, :]
            nc.tensor.matmul(
                bc, ones_sb[64:65, :],
                drow[64:65, h * _QC:(h + 1) * _QC],
                start=True, stop=True)
            otn = otcp.tile([64, _QC], f32r, tag="otn", bufs=3,
                            name=f"otn{qc}_{h}")
            nc.vector.tensor_mul(otn, otcs[h][0:64, :], bc[:, :])
            # place normalized O^T at this head's partitions (DMA can
            # cross partition bases; compute engines cannot)
            nc.gpsimd.dma_start(out=oT_sb[pb:pb + 64, c, qs], in_=otn)

        def oproj_tile(st, j):
            """output projection for l-tile j of chunk st."""
            qc = st["qc"]
            lt = qc * (_QC // 128) + j
            for n in range(2):
                op = scr.tile([128, 512], f32, tag="scr",
                              name=f"op{qc}_{j}_{n}")
                for cc in range(2):
                    nc.tensor.matmul(
                        op, oT_sb[:, cc, lt * 128:(lt + 1) * 128],
                        wo_sb[:, cc, n * 512:(n + 1) * 512],
                        start=(cc == 0), stop=(cc == 1))
                ob = otcp.tile([128, 512], f32, tag="ob", bufs=3)
                nc.vector.tensor_copy(ob, op)
                nc.sync.dma_start(
                    out=y[lt * 128:(lt + 1) * 128,
                          n * 512:(n + 1) * 512],
                    in_=ob)

        prev = None
        for lc in range(_NLC):
            # ---- projections + RoPE for l-chunk lc ----
            ls = slice(lc * _LC, (lc + 1) * _LC)
            if lc in xts:
                xt = xts.pop(lc)
            else:
                xt = xtp.tile([128, _NKC, _LC], f32r, tag="xt",
                              name=f"xt{lc}")
                for kc in range(_NKC):
                    nc.sync.dma_start(
                        out=xt[:, kc, :],
                        in_=xT_r[:, kc, lc * _LC:(lc + 1) * _LC])
            for w_sb, dst in ((wq_sb, qT_sb), (wk_sb, kT_sb)):
                for c in range(2):
                    ps = scr.tile([128, _LC], f32, tag="scr",
                                  name=f"ps{lc}_{c}")
                    for kc in range(_NKC):
                        nc.tensor.matmul(
                            ps, w_sb[:, kc, c * 128:(c + 1) * 128],
                            xt[:, kc, :],
                            start=(kc == 0), stop=(kc == _NKC - 1))
                    nc.vector.tensor_copy(dst[:, c, ls], ps)
            # RoPE (in place on this l-chunk): x*cos + (perm@x)*sin
            for dst in (qT_sb, kT_sb):
                for c in range(2):
                    rp = scr.tile([128, _LC], f32, tag="scr",
                                  name=f"rp{lc}_{c}")
                    nc.tensor.matmul(rp, perm_sb[:, :], dst[:, c, ls],
                                     start=True, stop=True)
                    tmp = rtmp.tile([128, _LC], f32, tag="rt")
                    nc.vector.tensor_mul(tmp, rp, sn_sb[:, ls])
                    nc.vector.tensor_mul(dst[:, c, ls], dst[:, c, ls],
                                         cs_sb[:, ls])
                    nc.vector.tensor_add(dst[:, c, ls], dst[:, c, ls], tmp)
            for j in range(_LC // 128):
                lt = lc * (_LC // 128) + j
                pv = scr.tile([128, _EG], f32, tag="scr", name=f"pv{lt}")
                for kc in range(_NKC):
                    nc.tensor.matmul(
                        pv, xt[:, kc, j * 128:(j + 1) * 128],
                        wv_sb[:, kc, :],
                        start=(kc == 0), stop=(kc == _NKC - 1))
                nc.vector.tensor_copy(
                    v_sb[:, lt, :, :_HD],
                    pv.rearrange("p (h e) -> p h e", h=_HPG))

            if debug_taps and lc == _NLC - 1:
                nc.sync.dma_start(out=taps["dq"][:, :, :], in_=qT_sb[:, :, :])
                nc.sync.dma_start(out=taps["dk"][:, :, :], in_=kT_sb[:, :, :])
                nc.sync.dma_start(out=taps["dv"][:, :, :, :],
                                  in_=v_sb[:, :, :, :_HD + 1])

            # ---- attention for q-chunk qc == lc ----
            qc = lc
            q0 = qc * _QC
            qs = slice(q0, q0 + _QC)
            nkt = (qc + 1) * (_QC // 128)
            # denominators land transposed [128, 4] per head so the
            # reciprocal is partition-parallel (free size 16, not 2048)
            dsb = nrm.tile([128, _HPG * 4], f32, tag="dsb")
            otcs = []
            for h in range(_HPG):
                c, pb = h // 2, 64 * (h % 2)
                ot = ops.tile([_HD + 1, _QC], f32, tag="ot")
                ngr = (nkt + _GK - 1) // _GK
                for g in range(ngr):
                    kts = list(range(g * _GK, min((g + 1) * _GK, nkt)))
                    sp = sps.tile([128, _GK * _QC], f32, tag="sp")
                    for i, kt in enumerate(kts):
                        nc.tensor.matmul(
                            sp[:, i * _QC:(i + 1) * _QC],
                            kT_sb[pb:pb + 64, c, kt * 128:(kt + 1) * 128],
                            qT_sb[pb:pb + 64, c, qs],
                            start=True, stop=True)
                    pt = ptp.tile([128, _GK * _QC], bf16, tag="pt")
                    na = len(kts) * _QC
                    nc.scalar.activation(pt[:, :na], sp[:, :na], EXP,
                                         scale=0.125)
                    if debug_taps and (qc, h, g) == (1, 0, 0):
                        nc.sync.dma_start(out=taps["dpt"][:, :],
                                          in_=pt[:, :])
                    for i, kt in enumerate(kts):
                        dj = kt - qc * (_QC // 128)
                        if dj >= 0:
                            nc.vector.tensor_mul(
                                pt[:, i * _QC:(i + 1) * _QC],
                                pt[:, i * _QC:(i + 1) * _QC],
                                msk_sb[:, dj, :])
                    for i, kt in enumerate(kts):
                        nc.tensor.matmul(
                            ot, v_sb[:, kt, h, :_HD + 1],
                            pt[:, i * _QC:(i + 1) * _QC],
                            start=(kt == 0), stop=(kt == nkt - 1),
                            skip_group_check=True)
                # free the psum bank quickly: copy numerator+denominator
                # to SBUF on ACT, stash the den row (transposed to
                # [128, 4]) via SBUF-to-SBUF DMA
                otc = otcp.tile([_HD + 1, _QC], f32, tag="otc",
                                name=f"otc{qc}_{h}")
                nc.scalar.copy(otc, ot[:, :])
                nc.gpsimd.dma_start(
                    out=dsb[:, h * 4:(h + 1) * 4],
                    in_=otc[64:65, :])
                otcs.append(otc)
                # interleave deferred work of the previous q-chunk to
                # keep the PE stream dense while ACT runs the exps
                if prev is not None:
                    if h == 0:
                        norm_head(prev, 0)
                        norm_head(prev, 1)
                    elif h == 1:
                        norm_head(prev, 2)
                        norm_head(prev, 3)
                        oproj_tile(prev, 0)
                    elif h == 2:
                        oproj_tile(prev, 1)
                        oproj_tile(prev, 2)
                    else:
                        oproj_tile(prev, 3)
            # reciprocal (partition-parallel) + fp32r rounding + row-
            # layout restore for the broadcast matmuls
            drec = nrm.tile([128, _HPG * 4], f32, tag="drec")
            nc.vector.reciprocal(drec, dsb)
            drecr = nrm.tile([128, _HPG * 4], f32r, tag="drecr")
            nc.scalar.copy(drecr, drec)
            drow = nrm.tile([65, _HPG * _QC], f32r, tag="drow", bufs=1)
            for h in range(_HPG):
                nc.gpsimd.dma_start(
                    out=drow[64:65, h * _QC:(h + 1) * _QC],
                    in_=drecr[:, h * 4:(h + 1) * 4])
            if debug_taps:
                nc.sync.dma_start(out=taps["dden"][qc:qc + 1, :],
                                  in_=dsb[:, :])
                nc.sync.dma_start(out=taps["drc"][qc:qc + 1, :],
                                  in_=drow[64:65, :])
            prev = {"qc": qc, "qs": qs, "otcs": otcs, "drow": drow}
        # tail: last q-chunk's normalization + projection
        for hh in range(_HPG):
            norm_head(prev, hh)
        for j in range(_QC // 128):
            oproj_tile(prev, j)
        if debug_taps:
            nc.sync.dma_start(out=taps["doT"][:, :, :],
                              in_=oT_sb[:, :, :])
    nc.compile()
    return nc


def get_nc(debug_taps=False):
    key = ("nc", debug_taps)
    if key not in _CACHE:
        _CACHE[key] = _build_nc(debug_taps)
    return _CACHE[key]


def make_in_maps(x, token_positions, Q, K, V, O_w):
    """Host-side sharding: per-core input dict (core = b*4 + hg)."""
    import ml_dtypes
    bf16 = ml_dtypes.bfloat16
    x = np.asarray(x, dtype=np.float32)
    tp = np.asarray(token_positions)
    Q = np.asarray(Q, dtype=np.float32)
    K = np.asarray(K, dtype=np.float32)
    V = np.asarray(V, dtype=np.float32)
    O_w = np.asarray(O_w, dtype=np.float32)

    # RoPE tables, [128, L]: rows 0..63 head-local e (cos repeated pairwise),
    # rows 64..127 a copy (two heads share one partition tile).
    i = np.arange(_HD // 2, dtype=np.float64)
    denom = _THETA ** (2.0 * i / _HD)                      # [32]
    ang = tp.astype(np.float64)[None, :] / denom[:, None]  # [32, L]
    cs64 = np.repeat(np.cos(ang), 2, axis=0)
    sn64 = np.repeat(np.sin(ang), 2, axis=0)
    cs = np.vstack([cs64, cs64]).astype(np.float32)
    sn = np.vstack([sn64, sn64]).astype(np.float32)

    # pairwise-rotation permutation (rot(x)[2i] = -x[2i+1], rot[2i+1] = x[2i])
    # as a stationary operand: out = permT.T @ x^T = Perm @ x^T
    p64 = np.zeros((64, 64), np.float32)
    for j in range(_HD // 2):
        p64[2 * j + 1, 2 * j] = -1.0
        p64[2 * j, 2 * j + 1] = 1.0
    permT = np.zeros((128, 128), np.float32)
    permT[0:64, 0:64] = p64
    permT[64:128, 64:128] = p64

    # causal masks for the 4 diagonal k-tiles of a 512-wide q-chunk
    pp = np.arange(128)[:, None]
    ff = np.arange(_QC)[None, :]
    msk = np.stack([(ff >= 128 * j + pp) for j in range(_QC // 128)],
                   axis=1).astype(bf16)                   # [128, 4, 512]

    Qr = Q.reshape(_H, _HD, _D)
    Kr = K.reshape(_H, _HD, _D)
    Vr = V.reshape(_H, _HD, _D)

    in_maps = []
    xT = [_tf32(x[b].T) for b in range(_B)]
    for core in range(_NCORES):
        b, hg = core // 4, core % 4
        hs = slice(hg * _HPG, (hg + 1) * _HPG)
        in_maps.append({
            "xT": xT[b],
            "wq": _tf32(Qr[hs].reshape(_EG, _D).T),
            "wk": _tf32(Kr[hs].reshape(_EG, _D).T),
            "wv": _tf32(Vr[hs].reshape(_EG, _D).T),
            "wo": _tf32(O_w[:, hg * _EG:(hg + 1) * _EG].T),
            "cs": cs, "sn": sn, "msk": msk, "perm": permT,
            "vones": np.ones((128, _HD), bf16),
            "onesr": np.ones((128, _HD), np.float32),
        })
    return in_maps


def run_on_hw(in_maps, trace=False, **kw):
    from concourse.bass_utils import run_bass_kernel_spmd
    nc = get_nc()
    return run_bass_kernel_spmd(nc, in_maps, core_ids=list(range(_NCORES)),
                                trace=trace, **kw)


def kernel(x, token_positions, Q, K, V, O_w):
    in_maps = make_in_maps(x, token_positions, Q, K, V, O_w)
    res = run_on_hw(in_maps)
    out = np.zeros((_B, _L, _D), dtype=np.float32)
    for core in range(_NCORES):
        out[core // 4] += res.results[core]["y"]
    return out



# revision 14
# speedup vs baseline: 1.0287x; 1.0287x over previous
"""Causal multi-head attention with RoPE on 8 Trainium2 NeuronCores.

Sharding: core = (batch b, head-group hg): b = core//4, hg = core%4.
Each core computes 4 heads of one batch element end-to-end (QKV projection,
RoPE, causal softmax attention, output-projection partial) and the host sums
the 4 per-head-group partials per batch (the "all-reduce" of the O-proj).

Device-side design (per core), v2 -- fused software pipeline:
  xT   [1024, 2048]  x[b] transposed (d on partitions, 8 chunks of 128)
  q^T,k^T  [256, 2048] head-transposed, computed with W^T stationary
  v    [l, e] layout (bf16) with a ones column appended per head (the PV
  matmul's 65th output row is then the softmax denominator).
  Scores computed transposed: S^T[k, q] = k^T.T @ q^T in f32r (TF32).
  The two heads sharing a 128-partition column block (partitions 0-63 /
  64-127) issue their K=64 score matmuls back-to-back; tile_position is
  auto-derived from the base partitions so the PE runs them CONCURRENTLY
  in separate row-groups (row tiling) -> ~2x on the score path.
  exp on ACT with no max subtraction (scores bounded ~17); on diagonal
  k-tiles all work (scores / exp / mask / PV) is restricted to the
  not-fully-masked column range [128*dj, 512) via strided APs, and the
  remaining triangle is zeroed multiplicatively post-exp in bf16.
  P*V runs in bf16 (P in [0,1]-scale, errors cancel between numerator
  and denominator).

  The main loop is a fused pipeline keyed on attention q-chunks: while
  ACT churns the exp stream of q-chunk qc, the PE is fed independent
  filler work -- projections+RoPE of l-chunk qc+1 and the deferred
  normalization + output-projection of q-chunk qc-1.  This keeps the PE
  stream dense so the HAM clock gate stays at 8/8 (2.4 GHz); the v1
  kernel alternated phases and spent ~half the run throttled at 1.2 GHz.
"""

import numpy as np

_B, _L, _D, _H, _HD = 2, 2048, 1024, 16, 64
_HPG = 4              # heads per group (per core)
_EG = _HPG * _HD      # 256
_NCORES = 8
_THETA = 10000.0
_QC = 512             # q-chunk width
_NQC = _L // _QC      # 4
_NKC = _D // 128      # 8 contraction chunks for projections
_LC = 512             # l-chunk for projections
_NLC = _L // _LC

_CACHE = {}


def _tf32(a):
    """Round float32 array to TF32 (fp32r): RNE to 10-bit mantissa."""
    b = np.ascontiguousarray(a, dtype=np.float32).view(np.uint32)
    b = (b + np.uint32(0xFFF) + ((b >> np.uint32(13)) & np.uint32(1))) \
        & np.uint32(0xFFFFE000)
    return b.view(np.float32)


def _build_nc():
    from contextlib import ExitStack

    import concourse.mybir as mybir
    import concourse.tile as tile
    from concourse import bacc

    f32 = mybir.dt.float32
    f32r = mybir.dt.float32r
    bf16 = mybir.dt.bfloat16
    EXP = mybir.ActivationFunctionType.Exp

    nc = bacc.Bacc("TRN2", target_bir_lowering=False, debug=False,
                   enable_asserts=False)
    xT = nc.dram_tensor("xT", [_D, _L], f32r, kind="ExternalInput")
    wq = nc.dram_tensor("wq", [_D, _EG], f32r, kind="ExternalInput")
    wk = nc.dram_tensor("wk", [_D, _EG], f32r, kind="ExternalInput")
    wv = nc.dram_tensor("wv", [_D, _EG], f32r, kind="ExternalInput")
    wo = nc.dram_tensor("wo", [_EG, _D], f32r, kind="ExternalInput")
    cs = nc.dram_tensor("cs", [128, _L], f32, kind="ExternalInput")
    sn = nc.dram_tensor("sn", [128, _L], f32, kind="ExternalInput")
    trid = nc.dram_tensor("trid", [128, 2, _QC], bf16, kind="ExternalInput")
    perm = nc.dram_tensor("perm", [128, 128], f32r, kind="ExternalInput")
    vones = nc.dram_tensor("vones", [128, _HD], bf16, kind="ExternalInput")
    onesr = nc.dram_tensor("onesr", [128, _HD], f32r, kind="ExternalInput")
    y = nc.dram_tensor("y", [_L, _D], f32, kind="ExternalOutput")

    with tile.TileContext(nc) as tc, ExitStack() as ctx:
        persist = ctx.enter_context(tc.tile_pool(name="persist", bufs=1))
        qT_sb = persist.tile([128, 2, _L], f32r)
        kT_sb = persist.tile([128, 2, _L], f32r)
        v_sb = persist.tile([128, _L // 128, _HPG, _HD + 4], bf16)
        oT_sb = persist.tile([128, 2, _L], f32r)
        wo_sb = persist.tile([128, 2, _D], f32r)
        tri_sb = persist.tile([128, 2, _QC], bf16)
        ones_sb = persist.tile([128, _HD], f32r)
        wq_sb = persist.tile([128, _NKC, _EG], f32r)
        wk_sb = persist.tile([128, _NKC, _EG], f32r)
        wv_sb = persist.tile([128, _NKC, _EG], f32r)
        cs_sb = persist.tile([128, _L], f32)
        sn_sb = persist.tile([128, _L], f32)
        perm_sb = persist.tile([128, 128], f32r)

        xtp = ctx.enter_context(tc.tile_pool(name="xtp", bufs=2))
        rtmp = ctx.enter_context(tc.tile_pool(name="rtmp", bufs=2))
        ptp = ctx.enter_context(tc.tile_pool(name="ptp", bufs=4))
        nrm = ctx.enter_context(tc.tile_pool(name="nrm", bufs=5))
        otcp = ctx.enter_context(tc.tile_pool(name="otcp", bufs=6))
        onp = ctx.enter_context(tc.tile_pool(name="onp", bufs=2))
        obp = ctx.enter_context(tc.tile_pool(name="obp", bufs=2))
        # PSUM budget (8 banks): sps 2x2 + ops 2x1 + scr 2x1
        sps = ctx.enter_context(tc.tile_pool(name="sps", bufs=2, space="PSUM"))
        ops = ctx.enter_context(tc.tile_pool(name="ops", bufs=2, space="PSUM"))
        scr = ctx.enter_context(tc.tile_pool(name="scr", bufs=2, space="PSUM"))

        # --- input loads: critical-path first.  The first projection
        # matmul needs only (wq chunk 0, xt0 chunk 0), so those two head
        # the queue and the rest streams behind them. ---
        wq_r = wq.rearrange("(c p) e -> p c e", p=128)
        wk_r = wk.rearrange("(c p) e -> p c e", p=128)
        wv_r = wv.rearrange("(c p) e -> p c e", p=128)
        xT_r = xT.rearrange("(c p) l -> p c l", p=128)
        xts = {}
        for lc in range(2):
            xts[lc] = xtp.tile([128, _NKC, _LC], f32r, tag="xt",
                               name=f"xt{lc}")
        for kc in range(_NKC):
            nc.sync.dma_start(out=wq_sb[:, kc, :], in_=wq_r[:, kc, :])
            nc.sync.dma_start(out=xts[0][:, kc, :], in_=xT_r[:, kc, 0:_LC])
        for kc in range(_NKC):
            nc.sync.dma_start(out=wk_sb[:, kc, :], in_=wk_r[:, kc, :])
        nc.gpsimd.dma_start(out=perm_sb, in_=perm[:, :])
        nc.gpsimd.dma_start(out=cs_sb, in_=cs[:, :])
        nc.gpsimd.dma_start(out=sn_sb, in_=sn[:, :])
        for kc in range(_NKC):
            nc.sync.dma_start(out=wv_sb[:, kc, :], in_=wv_r[:, kc, :])
        nc.gpsimd.dma_start(out=tri_sb, in_=trid[:, :, :])
        nc.gpsimd.dma_start(out=ones_sb, in_=onesr[:, :])
        nc.gpsimd.dma_start(
            out=v_sb[:, :, :, _HD:_HD + 1],
            in_=vones.rearrange("p (a b) -> p a b",
                                a=_L // 128).unsqueeze(3))
        for kc in range(_NKC):
            nc.sync.dma_start(out=xts[1][:, kc, :],
                              in_=xT_r[:, kc, _LC:2 * _LC])
        nc.gpsimd.dma_start(out=wo_sb,
                            in_=wo.rearrange("(c p) d -> p c d", p=128))

        def load_xt(lc):
            xt = xts[lc] = xtp.tile([128, _NKC, _LC], f32r, tag="xt",
                                    name=f"xt{lc}")
            for kc in range(_NKC):
                nc.sync.dma_start(
                    out=xt[:, kc, :],
                    in_=xT_r[:, kc, lc * _LC:(lc + 1) * _LC])

        # ---------- unit emitters (each is one filler quantum) ----------
        def proj_qk_unit(lc, w_sb, dst, c):
            """one 128-row output tile of the q or k projection."""
            ls = slice(lc * _LC, (lc + 1) * _LC)
            xt = xts[lc]
            ps = scr.tile([128, _LC], f32, tag="scr", name=f"ps{lc}_{c}")
            for kc in range(_NKC):
                nc.tensor.matmul(
                    ps, w_sb[:, kc, c * 128:(c + 1) * 128],
                    xt[:, kc, :],
                    start=(kc == 0), stop=(kc == _NKC - 1))
            nc.vector.tensor_copy(dst[:, c, ls], ps)

        def rope_unit(lc, dst, c):
            """RoPE in place on dst[:, c, lc-chunk]: x*cos + (perm@x)*sin."""
            ls = slice(lc * _LC, (lc + 1) * _LC)
            rp = scr.tile([128, _LC], f32, tag="scr", name=f"rp{lc}_{c}")
            nc.tensor.matmul(rp, perm_sb[:, :], dst[:, c, ls],
                             start=True, stop=True)
            tmp = rtmp.tile([128, _LC], f32, tag="rt")
            nc.vector.tensor_mul(tmp, rp, sn_sb[:, ls])
            nc.vector.tensor_mul(dst[:, c, ls], dst[:, c, ls], cs_sb[:, ls])
            nc.vector.tensor_add(dst[:, c, ls], dst[:, c, ls], tmp)

        def proj_v_unit(lc, j):
            """one 128-l-row tile of the v projection."""
            lt = lc * (_LC // 128) + j
            xt = xts[lc]
            pv = scr.tile([128, _EG], f32, tag="scr", name=f"pv{lt}")
            for kc in range(_NKC):
                nc.tensor.matmul(
                    pv, xt[:, kc, j * 128:(j + 1) * 128],
                    wv_sb[:, kc, :],
                    start=(kc == 0), stop=(kc == _NKC - 1))
            nc.vector.tensor_copy(
                v_sb[:, lt, :, :_HD],
                pv.rearrange("p (h e) -> p h e", h=_HPG))

        def norm_head(st, h):
            """broadcast recip, normalize, place head h of chunk st."""
            qc, qs = st["qc"], st["qs"]
            c, pb = h // 2, 64 * (h % 2)
            drr = st["drr"][h]
            otc = st["otc"][h]
            # rank-1 broadcast: ones[1,64].T @ recip_row -> [64, 512]
            bc = scr.tile([128, _QC], f32, tag="scr", name=f"bc{qc}_{h}")
            bc = bc[0:64, :]
            nc.tensor.matmul(bc, ones_sb[64:65, :], drr[64:65, :],
                             start=True, stop=True)
            otn = onp.tile([64, _QC], f32r, tag="otn", name=f"otn{qc}_{h}")
            nc.vector.tensor_mul(otn, otc[0:64, :], bc[:, :])
            # place normalized O^T at this head's partitions (DMA can
            # cross partition bases; compute engines cannot)
            nc.gpsimd.dma_start(out=oT_sb[pb:pb + 64, c, qs], in_=otn)

        def oproj_tile(st, j):
            """output projection for l-tile j of chunk st."""
            qc = st["qc"]
            lt = qc * (_QC // 128) + j
            for n in range(2):
                op = scr.tile([128, 512], f32, tag="scr",
                              name=f"op{qc}_{j}_{n}")
                for cc in range(2):
                    nc.tensor.matmul(
                        op, oT_sb[:, cc, lt * 128:(lt + 1) * 128],
                        wo_sb[:, cc, n * 512:(n + 1) * 512],
                        start=(cc == 0), stop=(cc == 1))
                ob = obp.tile([128, 512], f32, tag="ob")
                nc.vector.tensor_copy(ob, op)
                nc.sync.dma_start(
                    out=y[lt * 128:(lt + 1) * 128,
                          n * 512:(n + 1) * 512],
                    in_=ob)

        # ---------- the fused pipeline ----------
        # prologue: projections + RoPE for l-chunk 0
        for c in range(2):
            proj_qk_unit(0, wq_sb, qT_sb, c)
        for c in range(2):
            proj_qk_unit(0, wk_sb, kT_sb, c)
        for c in range(2):
            rope_unit(0, qT_sb, c)
        for c in range(2):
            rope_unit(0, kT_sb, c)
        for j in range(_LC // 128):
            proj_v_unit(0, j)

        prev = None            # deferred state of q-chunk qc-1
        for qc in range(_NQC):
            q0 = qc * _QC
            qs = slice(q0, q0 + _QC)
            nkt = (qc + 1) * (_QC // 128)

            # filler units for this phase, in consumption order
            fillers = []
            if prev is not None:
                st = prev
                fillers.append(lambda st=st: norm_head(st, 0))
                fillers.append(lambda st=st: norm_head(st, 1))
                fillers.append(lambda st=st: norm_head(st, 2))
                fillers.append(lambda st=st: norm_head(st, 3))
            lcn = qc + 1
            if lcn < _NLC:
                if lcn not in xts:
                    load_xt(lcn)
                if lcn + 1 < _NLC and (lcn + 1) not in xts:
                    load_xt(lcn + 1)
                for c in range(2):
                    fillers.append(
                        lambda c=c: proj_qk_unit(lcn, wq_sb, qT_sb, c))
                if prev is not None:
                    fillers.append(lambda st=prev: oproj_tile(st, 0))
                    fillers.append(lambda st=prev: oproj_tile(st, 1))
                for c in range(2):
                    fillers.append(
                        lambda c=c: proj_qk_unit(lcn, wk_sb, kT_sb, c))
                if prev is not None:
                    fillers.append(lambda st=prev: oproj_tile(st, 2))
                    fillers.append(lambda st=prev: oproj_tile(st, 3))
                for c in range(2):
                    fillers.append(lambda c=c: rope_unit(lcn, qT_sb, c))
                for c in range(2):
                    fillers.append(lambda c=c: rope_unit(lcn, kT_sb, c))
                if qc < 2:
                    for j in range(_LC // 128):
                        fillers.append(lambda j=j: proj_v_unit(lcn, j))
            else:
                # qc == 3: v-projection of l-chunk 3 runs here.  PV k-tile
                # 12+j of pair 0 (step index 12+j) reads v l-tile 12+j, so
                # each unit carries an emission deadline.
                for j in range(_LC // 128):
                    fillers.append((12 + j, lambda j=j: proj_v_unit(3, j)))
                if prev is not None:
                    fillers.append(lambda st=prev: oproj_tile(st, 0))
                    fillers.append(lambda st=prev: oproj_tile(st, 1))
                    fillers.append(lambda st=prev: oproj_tile(st, 2))
                    fillers.append(lambda st=prev: oproj_tile(st, 3))

            # normalize fillers to (deadline, fn); deadline = step index
            # before which the unit must be emitted (None = flexible)
            fillers = [f if isinstance(f, tuple) else (None, f)
                       for f in fillers]

            cur = {"qc": qc, "qs": qs, "otc": [None] * _HPG,
                   "drr": [None] * _HPG}
            nfill = len(fillers)
            popped = 0
            nsteps = 2 * nkt
            step = 0
            for hp in range(2):
                h0, h1 = 2 * hp, 2 * hp + 1
                ot0 = ops.tile([_HD + 1, _QC], f32, tag="ot",
                               name=f"ot{qc}_{h0}")
                ot1 = ops.tile([_HD + 1, _QC], f32, tag="ot",
                               name=f"ot{qc}_{h1}")
                for kt in range(nkt):
                    # units whose deadline hits this step must go first
                    due = popped
                    for j in range(popped, nfill):
                        if fillers[j][0] is not None and fillers[j][0] <= step:
                            due = j + 1
                    while popped < due:
                        fillers[popped][1]()
                        popped += 1
                    dj = kt - qc * (_QC // 128)
                    f0 = 128 * dj if dj >= 0 else 0   # first valid column
                    nv = _QC - f0                     # valid width
                    sp = sps.tile([128, 2, _QC], f32, tag="sp",
                                  name=f"sp{qc}_{hp}_{kt}")
                    # two concurrent K=64 score matmuls (row tiling via
                    # base partitions 0 / 64)
                    nc.tensor.matmul(
                        sp[:, 0, f0:], kT_sb[0:64, hp, kt * 128:(kt + 1) * 128],
                        qT_sb[0:64, hp, q0 + f0:q0 + _QC],
                        start=True, stop=True)
                    nc.tensor.matmul(
                        sp[:, 1, f0:], kT_sb[64:128, hp, kt * 128:(kt + 1) * 128],
                        qT_sb[64:128, hp, q0 + f0:q0 + _QC],
                        start=True, stop=True)
                    pt = ptp.tile([128, 2, _QC], bf16, tag="pt",
                                  name=f"pt{qc}_{hp}_{kt}")
                    nc.scalar.activation(pt[:, :, f0:], sp[:, :, f0:], EXP,
                                         scale=0.125)
                    if dj >= 0:
                        nc.vector.tensor_mul(pt[:, :, f0:], pt[:, :, f0:],
                                             tri_sb[:, :, :nv])
                    nc.tensor.matmul(
                        ot0[:, f0:], v_sb[:, kt, h0, :_HD + 1],
                        pt[:, 0, f0:],
                        start=(kt == 0), stop=(kt == nkt - 1),
                        skip_group_check=True)
                    nc.tensor.matmul(
                        ot1[:, f0:], v_sb[:, kt, h1, :_HD + 1],
                        pt[:, 1, f0:],
                        start=(kt == 0), stop=(kt == nkt - 1),
                        skip_group_check=True)
                    step += 1
                    # qc == 3 pair 0 normalization runs inside pair 1
                    if qc == _NQC - 1 and hp == 1 and kt == 2:
                        norm_head(cur, 0)
                    if qc == _NQC - 1 and hp == 1 and kt == 4:
                        norm_head(cur, 1)
                    want = (nfill * step) // nsteps
                    while popped < want:
                        fillers[popped][1]()
                        popped += 1
                # pair epilogue: free the psum banks (copy numerator +
                # denominator row to SBUF), reciprocal of the den row
                for h, ot in ((h0, ot0), (h1, ot1)):
                    otc = otcp.tile([_HD + 1, _QC], f32, tag="otc",
                                    name=f"otc{qc}_{h}")
                    nc.vector.tensor_copy(otc, ot[:, :])
                    drr = nrm.tile([_HD + 1, _QC], f32r, tag="drr",
                                   name=f"drr{qc}_{h}")
                    with nc.allow_low_precision(reason="f32r recip, 10-bit "
                                                "mantissa as in v1 drecr"):
                        nc.vector.reciprocal(drr[64:65, :], otc[64:65, :])
                    cur["otc"][h] = otc
                    cur["drr"][h] = drr
            while popped < nfill:
                fillers[popped][1]()
                popped += 1
            prev = cur
        # tail: last q-chunk's heads 2/3 + its output projection
        norm_head(prev, 2)
        norm_head(prev, 3)
        for j in range(_QC // 128):
            oproj_tile(prev, j)
    nc.compile()
    return nc


def get_nc():
    if "nc" not in _CACHE:
        _CACHE["nc"] = _build_nc()
    return _CACHE["nc"]


def make_in_maps(x, token_positions, Q, K, V, O_w):
    """Host-side sharding: per-core input dict (core = b*4 + hg)."""
    import ml_dtypes
    bf16 = ml_dtypes.bfloat16
    x = np.asarray(x, dtype=np.float32)
    tp = np.asarray(token_positions)
    Q = np.asarray(Q, dtype=np.float32)
    K = np.asarray(K, dtype=np.float32)
    V = np.asarray(V, dtype=np.float32)
    O_w = np.asarray(O_w, dtype=np.float32)

    # RoPE tables, [128, L]: rows 0..63 head-local e (cos repeated pairwise),
    # rows 64..127 a copy (two heads share one partition tile).
    i = np.arange(_HD // 2, dtype=np.float64)
    denom = _THETA ** (2.0 * i / _HD)                      # [32]
    ang = tp.astype(np.float64)[None, :] / denom[:, None]  # [32, L]
    cs64 = np.repeat(np.cos(ang), 2, axis=0)
    sn64 = np.repeat(np.sin(ang), 2, axis=0)
    cs = np.vstack([cs64, cs64]).astype(np.float32)
    sn = np.vstack([sn64, sn64]).astype(np.float32)

    # pairwise-rotation permutation (rot(x)[2i] = -x[2i+1], rot[2i+1] = x[2i])
    # as a stationary operand: out = permT.T @ x^T = Perm @ x^T
    p64 = np.zeros((64, 64), np.float32)
    for j in range(_HD // 2):
        p64[2 * j + 1, 2 * j] = -1.0
        p64[2 * j, 2 * j + 1] = 1.0
    permT = np.zeros((128, 128), np.float32)
    permT[0:64, 0:64] = p64
    permT[64:128, 64:128] = p64

    # shifted-triangle causal mask, duplicated for the two packed heads:
    # for diagonal k-tile dj, columns [128*dj, 512) keep tri[p, f-128*dj]
    pp = np.arange(128)[:, None]
    gg = np.arange(_QC)[None, :]
    tri = (gg >= pp).astype(bf16)                          # [128, 512]
    trid = np.stack([tri, tri], axis=1)                    # [128, 2, 512]

    Qr = Q.reshape(_H, _HD, _D)
    Kr = K.reshape(_H, _HD, _D)
    Vr = V.reshape(_H, _HD, _D)

    in_maps = []
    xT = [_tf32(x[b].T) for b in range(_B)]
    for core in range(_NCORES):
        b, hg = core // 4, core % 4
        hs = slice(hg * _HPG, (hg + 1) * _HPG)
        in_maps.append({
            "xT": xT[b],
            "wq": _tf32(Qr[hs].reshape(_EG, _D).T),
            "wk": _tf32(Kr[hs].reshape(_EG, _D).T),
            "wv": _tf32(Vr[hs].reshape(_EG, _D).T),
            "wo": _tf32(O_w[:, hg * _EG:(hg + 1) * _EG].T),
            "cs": cs, "sn": sn, "trid": trid, "perm": permT,
            "vones": np.ones((128, _HD), bf16),
            "onesr": np.ones((128, _HD), np.float32),
        })
    return in_maps


def run_on_hw(in_maps, trace=False, **kw):
    from concourse.bass_utils import run_bass_kernel_spmd
    nc = get_nc()
    return run_bass_kernel_spmd(nc, in_maps, core_ids=list(range(_NCORES)),
                                trace=trace, **kw)


def kernel(x, token_positions, Q, K, V, O_w):
    in_maps = make_in_maps(x, token_positions, Q, K, V, O_w)
    res = run_on_hw(in_maps)
    out = np.zeros((_B, _L, _D), dtype=np.float32)
    for core in range(_NCORES):
        out[core // 4] += res.results[core]["y"]
    return out


# revision 29
# speedup vs baseline: 1.1877x; 1.1545x over previous
"""Causal multi-head attention with RoPE on 8 Trainium2 NeuronCores.

Sharding: core = (batch b, head-group hg): b = core//4, hg = core%4.
Each core computes 4 heads of one batch element end-to-end (QKV projection,
RoPE, causal softmax attention, output-projection partial) and the host sums
the 4 per-head-group partials per batch (the "all-reduce" of the O-proj).

Device-side design (per core), v2 -- fused software pipeline:
  xT   [1024, 2048]  x[b] transposed (d on partitions, 8 chunks of 128)
  q^T,k^T  [256, 2048] head-transposed, computed with W^T stationary
  v    [l, e] layout (bf16) with a ones column appended per head (the PV
  matmul's 65th output row is then the softmax denominator).
  Scores computed transposed: S^T[k, q] = k^T.T @ q^T in f32r (TF32).
  The two heads sharing a 128-partition column block (partitions 0-63 /
  64-127) issue their K=64 score matmuls back-to-back; tile_position is
  auto-derived from the base partitions so the PE runs them CONCURRENTLY
  in separate row-groups (row tiling) -> ~2x on the score path.
  exp on ACT with no max subtraction (scores bounded ~17); on diagonal
  k-tiles all work (scores / exp / mask / PV) is restricted to the
  not-fully-masked column range [128*dj, 512) via strided APs, and the
  remaining triangle is zeroed multiplicatively post-exp in bf16.
  P*V runs in bf16 (P in [0,1]-scale, errors cancel between numerator
  and denominator).

  The main loop is a fused pipeline keyed on attention q-chunks: while
  ACT churns the exp stream of q-chunk qc, the PE is fed independent
  filler work -- projections+RoPE of l-chunk qc+1 and the deferred
  normalization + output-projection of q-chunk qc-1.  This keeps the PE
  stream dense so the HAM clock gate stays at 8/8 (2.4 GHz); the v1
  kernel alternated phases and spent ~half the run throttled at 1.2 GHz.
"""

import numpy as np

_B, _L, _D, _H, _HD = 2, 2048, 1024, 16, 64
_HPG = 4              # heads per group (per core)
_EG = _HPG * _HD      # 256
_NCORES = 8
_THETA = 10000.0
_QC = 512             # q-chunk width
_NQC = _L // _QC      # 4
_NKC = _D // 128      # 8 contraction chunks for projections
_LC = 512             # l-chunk for projections
_NLC = _L // _LC

_CACHE = {}


def _tf32(a):
    """Round float32 array to TF32 (fp32r): RNE to 10-bit mantissa."""
    b = np.ascontiguousarray(a, dtype=np.float32).view(np.uint32)
    b = (b + np.uint32(0xFFF) + ((b >> np.uint32(13)) & np.uint32(1))) \
        & np.uint32(0xFFFFE000)
    return b.view(np.float32)


def _build_nc():
    from contextlib import ExitStack

    import concourse.mybir as mybir
    import concourse.tile as tile
    from concourse import bacc

    f32 = mybir.dt.float32
    f32r = mybir.dt.float32r
    bf16 = mybir.dt.bfloat16
    EXP = mybir.ActivationFunctionType.Exp

    nc = bacc.Bacc("TRN2", target_bir_lowering=False, debug=False,
                   enable_asserts=False)
    xT = nc.dram_tensor("xT", [_D, _L], f32r, kind="ExternalInput")
    wq = nc.dram_tensor("wq", [_D, _EG], f32r, kind="ExternalInput")
    wk = nc.dram_tensor("wk", [_D, _EG], f32r, kind="ExternalInput")
    wv = nc.dram_tensor("wv", [_D, _EG], f32r, kind="ExternalInput")
    wo = nc.dram_tensor("wo", [_EG, _D], f32r, kind="ExternalInput")
    cs = nc.dram_tensor("cs", [128, _L], f32, kind="ExternalInput")
    sn = nc.dram_tensor("sn", [128, _L], f32, kind="ExternalInput")
    trid = nc.dram_tensor("trid", [128, 2, 128], bf16, kind="ExternalInput")
    perm = nc.dram_tensor("perm", [128, 128], f32r, kind="ExternalInput")
    vones = nc.dram_tensor("vones", [128, _HD], bf16, kind="ExternalInput")
    y = nc.dram_tensor("y", [_L, _D], f32, kind="ExternalOutput")

    with tile.TileContext(nc) as tc, ExitStack() as ctx:
        persist = ctx.enter_context(tc.tile_pool(name="persist", bufs=1))
        qT_sb = persist.tile([128, 2, _L], f32r)
        kT_sb = persist.tile([128, 2, _L], f32r)
        v_sb = persist.tile([128, _L // 128, _HPG, _HD + 4], bf16)
        oT_sb = persist.tile([128, 2, _L], f32r)
        wo_sb = persist.tile([128, 2, _D], f32r)
        tri_sb = persist.tile([128, 2, 128], bf16)
        wq_sb = persist.tile([128, _NKC, _EG], f32r)
        wk_sb = persist.tile([128, _NKC, _EG], f32r)
        wv_sb = persist.tile([128, _NKC, _EG], f32r)
        cs_sb = persist.tile([128, _L], f32)
        sn_sb = persist.tile([128, _L], f32)
        perm_sb = persist.tile([128, 128], f32r)

        xtp = ctx.enter_context(tc.tile_pool(name="xtp", bufs=2))
        rtmp = ctx.enter_context(tc.tile_pool(name="rtmp", bufs=2))
        ptp = ctx.enter_context(tc.tile_pool(name="ptp", bufs=4))
        nrm = ctx.enter_context(tc.tile_pool(name="nrm", bufs=5))
        otcp = ctx.enter_context(tc.tile_pool(name="otcp", bufs=6))
        onp = ctx.enter_context(tc.tile_pool(name="onp", bufs=2))
        bcp = ctx.enter_context(tc.tile_pool(name="bcp", bufs=2))
        obp = ctx.enter_context(tc.tile_pool(name="obp", bufs=2))
        # PSUM budget (8 banks): sps 2x2 + ops 2x1 + scr 2x1
        sps = ctx.enter_context(tc.tile_pool(name="sps", bufs=2, space="PSUM"))
        ops = ctx.enter_context(tc.tile_pool(name="ops", bufs=2, space="PSUM"))
        scr = ctx.enter_context(tc.tile_pool(name="scr", bufs=2, space="PSUM"))

        # --- input loads: critical-path first.  The first projection
        # matmul needs only (wq chunk 0, xt0 chunk 0), so those two head
        # the queue and the rest streams behind them. ---
        wq_r = wq.rearrange("(c p) e -> p c e", p=128)
        wk_r = wk.rearrange("(c p) e -> p c e", p=128)
        wv_r = wv.rearrange("(c p) e -> p c e", p=128)
        xT_r = xT.rearrange("(c p) l -> p c l", p=128)
        xts = {}
        for lc in range(2):
            xts[lc] = xtp.tile([128, _NKC, _LC], f32r, tag="xt",
                               name=f"xt{lc}")
        for kc in range(_NKC):
            nc.sync.dma_start(out=wq_sb[:, kc, :], in_=wq_r[:, kc, :])
            nc.sync.dma_start(out=xts[0][:, kc, :], in_=xT_r[:, kc, 0:_LC])
        for kc in range(_NKC):
            nc.sync.dma_start(out=wk_sb[:, kc, :], in_=wk_r[:, kc, :])
        nc.gpsimd.dma_start(out=perm_sb, in_=perm[:, :])
        nc.gpsimd.dma_start(out=cs_sb, in_=cs[:, :])
        nc.gpsimd.dma_start(out=sn_sb, in_=sn[:, :])
        for kc in range(_NKC):
            nc.sync.dma_start(out=wv_sb[:, kc, :], in_=wv_r[:, kc, :])
        nc.gpsimd.dma_start(out=tri_sb, in_=trid[:, :, :])
        nc.gpsimd.dma_start(
            out=v_sb[:, :, :, _HD:_HD + 1],
            in_=vones.rearrange("p (a b) -> p a b",
                                a=_L // 128).unsqueeze(3))
        for kc in range(_NKC):
            nc.sync.dma_start(out=xts[1][:, kc, :],
                              in_=xT_r[:, kc, _LC:2 * _LC])
        nc.gpsimd.dma_start(out=wo_sb,
                            in_=wo.rearrange("(c p) d -> p c d", p=128))

        def load_xt(lc):
            xt = xts[lc] = xtp.tile([128, _NKC, _LC], f32r, tag="xt",
                                    name=f"xt{lc}")
            for kc in range(_NKC):
                nc.sync.dma_start(
                    out=xt[:, kc, :],
                    in_=xT_r[:, kc, lc * _LC:(lc + 1) * _LC])

        # ---------- unit emitters (each is one filler quantum) ----------
        def proj_qk_unit(lc, w_sb, dst, c):
            """one 128-row output tile of the q or k projection."""
            ls = slice(lc * _LC, (lc + 1) * _LC)
            xt = xts[lc]
            ps = scr.tile([128, _LC], f32, tag="scr", name=f"ps{lc}_{c}")
            for kc in range(_NKC):
                nc.tensor.matmul(
                    ps, w_sb[:, kc, c * 128:(c + 1) * 128],
                    xt[:, kc, :],
                    start=(kc == 0), stop=(kc == _NKC - 1))
            nc.vector.tensor_copy(dst[:, c, ls], ps)

        def rope_unit(lc, dst, c):
            """RoPE in place on dst[:, c, lc-chunk]: x*cos + (perm@x)*sin."""
            ls = slice(lc * _LC, (lc + 1) * _LC)
            rp = scr.tile([128, _LC], f32, tag="scr", name=f"rp{lc}_{c}")
            nc.tensor.matmul(rp, perm_sb[:, :], dst[:, c, ls],
                             start=True, stop=True)
            tmp = rtmp.tile([128, _LC], f32, tag="rt")
            nc.vector.tensor_mul(tmp, rp, sn_sb[:, ls])
            nc.vector.tensor_mul(dst[:, c, ls], dst[:, c, ls], cs_sb[:, ls])
            nc.vector.tensor_add(dst[:, c, ls], dst[:, c, ls], tmp)

        def proj_v_unit(lc, j):
            """one 128-l-row tile of the v projection."""
            lt = lc * (_LC // 128) + j
            xt = xts[lc]
            pv = scr.tile([128, _EG], f32, tag="scr", name=f"pv{lt}")
            for kc in range(_NKC):
                nc.tensor.matmul(
                    pv, xt[:, kc, j * 128:(j + 1) * 128],
                    wv_sb[:, kc, :],
                    start=(kc == 0), stop=(kc == _NKC - 1))
            nc.vector.tensor_copy(
                v_sb[:, lt, :, :_HD],
                pv.rearrange("p (h e) -> p h e", h=_HPG))

        def norm_head(st, h):
            """broadcast recip, normalize, place head h of chunk st."""
            qc, qs = st["qc"], st["qs"]
            c, pb = h // 2, 64 * (h % 2)
            drow = st["drow"][h // 2]
            otc = st["otc"][h]
            # broadcast the recip row across 64 partitions on GpSimd
            bc = bcp.tile([64, _QC], f32r, tag="bc", name=f"bc{qc}_{h}")
            nc.gpsimd.partition_broadcast(
                bc, drow[0:1, (h % 2) * _QC:(h % 2 + 1) * _QC])
            otn = onp.tile([64, _QC], f32r, tag="otn", name=f"otn{qc}_{h}")
            nc.vector.tensor_mul(otn, otc[0:64, :], bc[:, :])
            # place normalized O^T at this head's partitions (DMA can
            # cross partition bases; compute engines cannot)
            nc.gpsimd.dma_start(out=oT_sb[pb:pb + 64, c, qs], in_=otn)

        def oproj_tile(st, j):
            """output projection for l-tile j of chunk st."""
            qc = st["qc"]
            lt = qc * (_QC // 128) + j
            for n in range(2):
                op = scr.tile([128, 512], f32, tag="scr",
                              name=f"op{qc}_{j}_{n}")
                for cc in range(2):
                    nc.tensor.matmul(
                        op, oT_sb[:, cc, lt * 128:(lt + 1) * 128],
                        wo_sb[:, cc, n * 512:(n + 1) * 512],
                        start=(cc == 0), stop=(cc == 1))
                ob = obp.tile([128, 512], f32, tag="ob")
                nc.vector.tensor_copy(ob, op)
                nc.sync.dma_start(
                    out=y[lt * 128:(lt + 1) * 128,
                          n * 512:(n + 1) * 512],
                    in_=ob)

        # ---------- the fused pipeline ----------
        # prologue: projections + RoPE for l-chunk 0
        for c in range(2):
            proj_qk_unit(0, wq_sb, qT_sb, c)
        for c in range(2):
            proj_qk_unit(0, wk_sb, kT_sb, c)
        for c in range(2):
            rope_unit(0, qT_sb, c)
        for c in range(2):
            rope_unit(0, kT_sb, c)
        for j in range(_LC // 128):
            proj_v_unit(0, j)

        prev = None            # deferred state of q-chunk qc-1
        for qc in range(_NQC):
            q0 = qc * _QC
            qs = slice(q0, q0 + _QC)
            nkt = (qc + 1) * (_QC // 128)

            # filler units for this phase, in consumption order
            fillers = []
            if prev is not None:
                st = prev
                fillers.append(lambda st=st: norm_head(st, 0))
                fillers.append(lambda st=st: norm_head(st, 1))
                fillers.append(lambda st=st: norm_head(st, 2))
                fillers.append(lambda st=st: norm_head(st, 3))
            lcn = qc + 1
            if lcn < _NLC:
                if lcn not in xts:
                    load_xt(lcn)
                if lcn + 1 < _NLC and (lcn + 1) not in xts:
                    load_xt(lcn + 1)
                for c in range(2):
                    fillers.append(
                        lambda c=c: proj_qk_unit(lcn, wq_sb, qT_sb, c))
                if prev is not None:
                    fillers.append(lambda st=prev: oproj_tile(st, 0))
                    fillers.append(lambda st=prev: oproj_tile(st, 1))
                for c in range(2):
                    fillers.append(
                        lambda c=c: proj_qk_unit(lcn, wk_sb, kT_sb, c))
                if prev is not None:
                    fillers.append(lambda st=prev: oproj_tile(st, 2))
                    fillers.append(lambda st=prev: oproj_tile(st, 3))
                for c in range(2):
                    fillers.append(lambda c=c: rope_unit(lcn, qT_sb, c))
                for c in range(2):
                    fillers.append(lambda c=c: rope_unit(lcn, kT_sb, c))
                if qc < 2:
                    for j in range(_LC // 128):
                        fillers.append(lambda j=j: proj_v_unit(lcn, j))
            else:
                # qc == 3: v-projection of l-chunk 3 runs here.  PV k-tile
                # 12+j of pair 0 (step index 12+j) reads v l-tile 12+j, so
                # each unit carries an emission deadline.
                for j in range(_LC // 128):
                    fillers.append((12 + j, lambda j=j: proj_v_unit(3, j)))
                if prev is not None:
                    fillers.append(lambda st=prev: oproj_tile(st, 0))
                    fillers.append(lambda st=prev: oproj_tile(st, 1))
                    fillers.append(lambda st=prev: oproj_tile(st, 2))
                    fillers.append(lambda st=prev: oproj_tile(st, 3))

            # normalize fillers to (deadline, fn); deadline = step index
            # before which the unit must be emitted (None = flexible)
            fillers = [f if isinstance(f, tuple) else (None, f)
                       for f in fillers]

            cur = {"qc": qc, "qs": qs, "otc": [None] * _HPG,
                   "drow": [None, None]}
            nfill = len(fillers)
            popped = 0
            nsteps = 2 * nkt
            step = 0
            for hp in range(2):
                h0, h1 = 2 * hp, 2 * hp + 1
                ot0 = ops.tile([_HD + 1, _QC], f32, tag="ot",
                               name=f"ot{qc}_{h0}")
                ot1 = ops.tile([_HD + 1, _QC], f32, tag="ot",
                               name=f"ot{qc}_{h1}")
                for kt in range(nkt):
                    # units whose deadline hits this step must go first
                    due = popped
                    for j in range(popped, nfill):
                        if fillers[j][0] is not None and fillers[j][0] <= step:
                            due = j + 1
                    while popped < due:
                        fillers[popped][1]()
                        popped += 1
                    dj = kt - qc * (_QC // 128)
                    f0 = 128 * dj if dj >= 0 else 0   # first valid column
                    nv = _QC - f0                     # valid width
                    sp = sps.tile([128, 2, _QC], f32, tag="sp",
                                  name=f"sp{qc}_{hp}_{kt}")
                    # two concurrent K=64 score matmuls (row tiling via
                    # base partitions 0 / 64)
                    nc.tensor.matmul(
                        sp[:, 0, f0:], kT_sb[0:64, hp, kt * 128:(kt + 1) * 128],
                        qT_sb[0:64, hp, q0 + f0:q0 + _QC],
                        start=True, stop=True)
                    nc.tensor.matmul(
                        sp[:, 1, f0:], kT_sb[64:128, hp, kt * 128:(kt + 1) * 128],
                        qT_sb[64:128, hp, q0 + f0:q0 + _QC],
                        start=True, stop=True)
                    pt = ptp.tile([128, 2, _QC], bf16, tag="pt",
                                  name=f"pt{qc}_{hp}_{kt}")
                    nc.scalar.activation(pt[:, :, f0:], sp[:, :, f0:], EXP,
                                         scale=0.125)
                    if dj >= 0:
                        # only the 128-wide diagonal block needs masking;
                        # columns beyond it are fully valid
                        nc.vector.tensor_mul(pt[:, :, f0:f0 + 128],
                                             pt[:, :, f0:f0 + 128],
                                             tri_sb[:, :, :])
                    nc.tensor.matmul(
                        ot0[:, f0:], v_sb[:, kt, h0, :_HD + 1],
                        pt[:, 0, f0:],
                        start=(kt == 0), stop=(kt == nkt - 1),
                        skip_group_check=True)
                    nc.tensor.matmul(
                        ot1[:, f0:], v_sb[:, kt, h1, :_HD + 1],
                        pt[:, 1, f0:],
                        start=(kt == 0), stop=(kt == nkt - 1),
                        skip_group_check=True)
                    step += 1
                    # qc == 3 pair 0 normalization runs inside pair 1
                    if qc == _NQC - 1 and hp == 1 and kt == 2:
                        norm_head(cur, 0)
                    if qc == _NQC - 1 and hp == 1 and kt == 4:
                        norm_head(cur, 1)
                    want = (nfill * step) // nsteps
                    while popped < want:
                        fillers[popped][1]()
                        popped += 1
                # pair epilogue: free the psum banks (copy numerator +
                # denominator row to SBUF); den rows land transposed in
                # [128, 4] blocks so the reciprocal is partition-parallel
                dsb = nrm.tile([128, 2 * 4], f32, tag="dsb",
                               name=f"dsb{qc}_{hp}")
                for i, (h, ot) in enumerate(((h0, ot0), (h1, ot1))):
                    otc = otcp.tile([_HD + 1, _QC], f32, tag="otc",
                                    name=f"otc{qc}_{h}")
                    nc.vector.tensor_copy(otc, ot[:, :])
                    nc.gpsimd.dma_start(out=dsb[:, i * 4:(i + 1) * 4],
                                        in_=otc[64:65, :])
                    cur["otc"][h] = otc
                drecr = nrm.tile([128, 2 * 4], f32r, tag="drecr",
                                 name=f"drecr{qc}_{hp}")
                with nc.allow_low_precision(reason="f32r recip, 10-bit "
                                            "mantissa, errors cancel"):
                    nc.vector.reciprocal(drecr, dsb)
                drow = nrm.tile([1, 2 * _QC], f32r, tag="drow",
                                name=f"drow{qc}_{hp}")
                for i in range(2):
                    nc.gpsimd.dma_start(
                        out=drow[0:1, i * _QC:(i + 1) * _QC],
                        in_=drecr[:, i * 4:(i + 1) * 4])
                cur["drow"][hp] = drow
            while popped < nfill:
                fillers[popped][1]()
                popped += 1
            prev = cur
        # tail: last q-chunk's heads 2/3 + its output projection
        norm_head(prev, 2)
        norm_head(prev, 3)
        for j in range(_QC // 128):
            oproj_tile(prev, j)
    nc.compile()
    return nc


def get_nc():
    if "nc" not in _CACHE:
        _CACHE["nc"] = _build_nc()
    return _CACHE["nc"]


def make_in_maps(x, token_positions, Q, K, V, O_w):
    """Host-side sharding: per-core input dict (core = b*4 + hg)."""
    import ml_dtypes
    bf16 = ml_dtypes.bfloat16
    x = np.asarray(x, dtype=np.float32)
    tp = np.asarray(token_positions)
    Q = np.asarray(Q, dtype=np.float32)
    K = np.asarray(K, dtype=np.float32)
    V = np.asarray(V, dtype=np.float32)
    O_w = np.asarray(O_w, dtype=np.float32)

    # RoPE tables, [128, L]: rows 0..63 head-local e (cos repeated pairwise),
    # rows 64..127 a copy (two heads share one partition tile).
    i = np.arange(_HD // 2, dtype=np.float64)
    denom = _THETA ** (2.0 * i / _HD)                      # [32]
    ang = tp.astype(np.float64)[None, :] / denom[:, None]  # [32, L]
    cs64 = np.repeat(np.cos(ang), 2, axis=0)
    sn64 = np.repeat(np.sin(ang), 2, axis=0)
    cs = np.vstack([cs64, cs64]).astype(np.float32)
    sn = np.vstack([sn64, sn64]).astype(np.float32)

    # pairwise-rotation permutation (rot(x)[2i] = -x[2i+1], rot[2i+1] = x[2i])
    # as a stationary operand: out = permT.T @ x^T = Perm @ x^T
    p64 = np.zeros((64, 64), np.float32)
    for j in range(_HD // 2):
        p64[2 * j + 1, 2 * j] = -1.0
        p64[2 * j, 2 * j + 1] = 1.0
    permT = np.zeros((128, 128), np.float32)
    permT[0:64, 0:64] = p64
    permT[64:128, 64:128] = p64

    # shifted-triangle causal mask, duplicated for the two packed heads:
    # for diagonal k-tile dj, columns [128*dj, 512) keep tri[p, f-128*dj]
    pp = np.arange(128)[:, None]
    gg = np.arange(128)[None, :]
    tri = (gg >= pp).astype(bf16)                          # [128, 128]
    trid = np.stack([tri, tri], axis=1)                    # [128, 2, 128]

    Qr = Q.reshape(_H, _HD, _D)
    Kr = K.reshape(_H, _HD, _D)
    Vr = V.reshape(_H, _HD, _D)

    in_maps = []
    xT = [_tf32(x[b].T) for b in range(_B)]
    for core in range(_NCORES):
        b, hg = core // 4, core % 4
        hs = slice(hg * _HPG, (hg + 1) * _HPG)
        in_maps.append({
            "xT": xT[b],
            "wq": _tf32(Qr[hs].reshape(_EG, _D).T),
            "wk": _tf32(Kr[hs].reshape(_EG, _D).T),
            "wv": _tf32(Vr[hs].reshape(_EG, _D).T),
            "wo": _tf32(O_w[:, hg * _EG:(hg + 1) * _EG].T),
            "cs": cs, "sn": sn, "trid": trid, "perm": permT,
            "vones": np.ones((128, _HD), bf16),
        })
    return in_maps


def run_on_hw(in_maps, trace=False, **kw):
    from concourse.bass_utils import run_bass_kernel_spmd
    nc = get_nc()
    return run_bass_kernel_spmd(nc, in_maps, core_ids=list(range(_NCORES)),
                                trace=trace, **kw)


def kernel(x, token_positions, Q, K, V, O_w):
    in_maps = make_in_maps(x, token_positions, Q, K, V, O_w)
    res = run_on_hw(in_maps)
    out = np.zeros((_B, _L, _D), dtype=np.float32)
    for core in range(_NCORES):
        out[core // 4] += res.results[core]["y"]
    return out
